# revision 7
# baseline (speedup 1.0000x reference)
"""MoE top-2 routed linear (nn_MoELinear) on 8 Trainium2 NeuronCores.

Strategy (load-balanced expert parallelism):
  - Gating (tiny: [N,1024]x[1024,8] matmul + top-2 + softmax) runs on host
    with jax-CPU, replicating the reference op-for-op so the top-2 decisions
    match the reference bitwise.
  - The per-expert work is split into "jobs" = (expert, 2048-col output
    half).  Each core holds up to 4 resident W segments of [1024, 2048]
    (fp16) in SBUF; a job is assigned to one or more (core, segment) cells,
    and its token tiles (128 tokens each) fill that cell's slot range.  A
    small search over segment capacities packs all jobs into the minimum
    uniform slot count MT per core (~ceil(2*sum(M_e)/8)), eliminating the
    load imbalance of the one-expert-per-core layout.
  - Device computes y[slot] = (x_tile @ Wseg) * gate_w in fp16 (fp32 PSUM
    accumulation), which measured at ~2e-3 max rel err for bf16 and better
    for fp16 -- far inside the 2e-2 gate.  y is written back in fp16.
  - Host scatters/accumulates the per-slot outputs into the final fp32
    result.
"""

import os

import numpy as np

NUM_CORES = 8
NUM_EXPERTS = 8
TOP_K = 2
P = 128  # partitions
N_TILE = 512  # psum free-dim tile (one bank of fp32)
SEG_COLS = 2048  # output columns per W segment (half of d_out)
MAX_SEGS = 4

# enable NTFF tracing (sets LAST_RUN_INFO["exec_time_ns"])
TRACE = os.environ.get("MOE_TRACE", "0") == "1"
MM_DTYPE = "float16"

LAST_RUN_INFO = {}
_NC_CACHE = {}


def _routing(x_flat, Wg, bg):
    """Replicate the reference gating bitwise on jax-CPU; numpy fallback."""
    try:
        import jax
        import jax.numpy as jnp

        with jax.default_device(jax.devices("cpu")[0]):
            xf = jnp.asarray(x_flat)
            gate_logits = xf @ jnp.asarray(Wg).T + jnp.asarray(bg)
            top_w, top_idx = jax.lax.top_k(gate_logits, TOP_K)
            top_w = jax.nn.softmax(top_w, axis=-1)
            return np.asarray(top_idx), np.asarray(top_w)
    except Exception:
        logits = x_flat @ Wg.T + bg
        top_idx = np.argsort(-logits, axis=1, kind="stable")[:, :TOP_K]
        top_v = np.take_along_axis(logits, top_idx, axis=1)
        e = np.exp(top_v - top_v.max(axis=1, keepdims=True))
        top_w = e / e.sum(axis=1, keepdims=True)
        return top_idx, top_w.astype(np.float32)


def _try_assign(jobs, caps):
    """Greedy pack of jobs (size, e, h) into cells of capacities caps x 8.

    Returns list of (e, h, tile_lo, tile_hi, core, seg) cell fills, or None.
    A job may span several cells (its tiles are split across them)."""
    cells = []  # [remaining?, core, seg, cap]
    for s, q in enumerate(caps):
        if q <= 0:
            continue
        for core in range(NUM_CORES):
            cells.append([q, core, s, False])  # cap, core, seg, used
    fills = []
    for size, e, h in jobs:
        free = [c for c in cells if not c[3]]
        # smallest single cell that fits
        fit = None
        for c in sorted(free, key=lambda c: c[0]):
            if c[0] >= size:
                fit = c
                break
        take = []
        if fit is not None:
            take = [fit]
        else:
            got = 0
            for c in sorted(free, key=lambda c: -c[0]):
                if c in take:
                    continue
                # if a single remaining cell can finish the cover, use the
                # smallest such cell instead of the next-largest
                fin = None
                for c2 in sorted(free, key=lambda c: c[0]):
                    if c2 not in take and c2[0] >= size - got:
                        fin = c2
                        break
                if fin is not None:
                    take.append(fin)
                    got += fin[0]
                    break
                take.append(c)
                got += c[0]
                if got >= size:
                    break
            if got < size:
                return None
        lo = 0
        for c in take:
            c[3] = True
            n = min(c[0], size - lo)
            fills.append((e, h, lo, lo + n, c[1], c[2]))
            lo += n
    return fills


def _plan(M):
    """Choose slot count MT, segment capacities, and the job->cell packing."""
    jobs = []
    for e in range(NUM_EXPERTS):
        if M[e] > 0:
            jobs.append((M[e], e, 0))
            jobs.append((M[e], e, 1))
    jobs.sort(reverse=True)
    total = sum(s for s, _, _ in jobs)
    lb = max(1, -(-total // NUM_CORES))
    for mt in range(lb, 4 * 64 + 1):
        for a in range(-(-mt // MAX_SEGS), mt + 1):
            for b in range(0, min(a, mt - a) + 1):
                rem = mt - a - b
                for c in range(max(0, rem - b), min(b, rem) + 1):
                    dd = rem - c
                    if dd > c:
                        continue
                    caps = (a, b, c, dd)
                    fills = _try_assign(jobs, caps)
                    if fills is not None:
                        return mt, caps, fills
    raise RuntimeError("no feasible packing")


def _build_program(MT, caps, CIN):
    """Uniform per-core program: MT slots, slot m uses W segment seg(m).

    y[m] = (x[m] @ Wseg) * sc[:, m] for its 2048 columns, fp16 I/O."""
    import concourse.mybir as mybir
    import concourse.tile as tile
    from concourse import bacc

    f32 = mybir.dt.float32
    f16 = mybir.dt.float16

    KT = CIN // P  # 8
    NT = SEG_COLS // N_TILE  # 4
    nseg = sum(1 for q in caps if q > 0)
    seg_of_slot = []
    for s, q in enumerate(caps):
        seg_of_slot += [s] * q

    nc = bacc.Bacc()
    # xt[m, p, k*128+j] = x[token (m,j), cin (k*128+p)]  (lhsT pretiled)
    xt = nc.declare_dram_parameter("xt", [MT, P, CIN], f16, isOutput=False)
    # wt[s, n, p, k, c] = W_seg_s.T[k*128+p, n*512+c]
    wt = nc.declare_dram_parameter(
        "wt", [nseg, NT, P, KT, N_TILE], f16, isOutput=False
    )
    # sc[p, m] = gate weight of token (m, p)
    sc = nc.declare_dram_parameter("sc", [P, MT], f32, isOutput=False)
    y = nc.declare_dram_parameter("y", [MT, P, SEG_COLS], f16, isOutput=True)

    PF = 6  # x-tile prefetch depth

    with tile.TileContext(nc) as tc:
        with (
            tc.tile_pool(name="wpool", bufs=1) as wpool,
            tc.tile_pool(name="xpool", bufs=min(PF + 2, MT)) as xpool,
            tc.tile_pool(name="spool", bufs=1) as spool,
            tc.tile_pool(name="opool", bufs=8) as opool,
            tc.tile_pool(name="pspool", bufs=8, space="PSUM") as pspool,
        ):
            # scales: one tiny contiguous DMA up front
            sctile = spool.tile([P, MT], f32)
            nc.sync.dma_start(out=sctile[:], in_=sc[:, :])

            # W segments as 3D tiles [p, k, 2048]; one 1MB descriptor per
            # (seg, n-quarter) keeps trigger count low (the trigger itself
            # costs ~600ns of engine time), except the very first n-group
            # which is split per-k so the first psum group can start after
            # ~128KB instead of 1MB.
            wtiles = [
                wpool.tile([P, KT, SEG_COLS], f16, tag=f"w{s}", name=f"w{s}")
                for s in range(nseg)
            ]

            def load_w(s, n, split=False):
                if split:
                    for k in range(KT):
                        nc.sync.dma_start(
                            out=wtiles[s][:, k, n * N_TILE : (n + 1) * N_TILE],
                            in_=wt[s, n, :, k],
                        )
                else:
                    nc.sync.dma_start(
                        out=wtiles[s][:, :, n * N_TILE : (n + 1) * N_TILE],
                        in_=wt[s, n],
                    )

            def load_xm(m):
                xtile = xpool.tile([P, CIN], f16, name="xtile", tag="xtile")
                nc.sync.dma_start(out=xtile[:], in_=xt[m])
                return xtile

            # issue order interleaves W and x by need-by time
            load_w(0, 0, split=True)
            pending = [load_xm(0), load_xm(1)]
            for n in range(1, NT):
                load_w(0, n)
            for m in range(2, min(PF, MT)):
                pending.append(load_xm(m))
            for s in range(1, nseg):
                for n in range(NT):
                    load_w(s, n)

            ydma = nc.gpsimd

            for m in range(MT):
                s = seg_of_slot[m]
                xtile = pending.pop(0)
                if m + PF < MT:
                    pending.append(load_xm(m + PF))
                stile = sctile[:, m : m + 1]
                for n in range(NT):
                    psum = pspool.tile([P, N_TILE], f32)
                    for k in range(KT):
                        nc.tensor.matmul(
                            psum[:],
                            lhsT=xtile[:, k * P : (k + 1) * P],
                            rhs=wtiles[s][:, k, n * N_TILE : (n + 1) * N_TILE],
                            start=(k == 0),
                            stop=(k == KT - 1),
                        )
                    otile = opool.tile([P, N_TILE], f16)
                    # alternate eviction engines: halves the serial latency
                    # of draining a slot's four psum banks
                    if n % 2 == 0:
                        nc.scalar.activation(
                            otile[:],
                            psum[:],
                            mybir.ActivationFunctionType.Copy,
                            scale=stile,
                        )
                    else:
                        nc.vector.tensor_scalar_mul(otile[:], psum[:], stile)
                    ydma.dma_start(
                        out=y[m, :, n * N_TILE : (n + 1) * N_TILE],
                        in_=otile[:],
                    )
    nc.finalize()
    return nc


def kernel(x, We, Wg, bg):
    from concourse.bass_utils import run_bass_kernel_spmd

    B, T, CIN = x.shape
    E, DOUT, _ = We.shape
    N = B * T
    x_flat = np.ascontiguousarray(x.reshape(N, CIN), dtype=np.float32)

    top_idx, top_w = _routing(x_flat, Wg, bg)

    # dispatch: token lists per expert
    idx_e, w_e, M = [], [], []
    for e in range(E):
        sel0 = top_idx[:, 0] == e
        sel1 = top_idx[:, 1] == e
        rows = np.nonzero(sel0 | sel1)[0]
        w = np.where(sel0[rows], top_w[rows, 0], top_w[rows, 1]).astype(np.float32)
        idx_e.append(rows)
        w_e.append(w)
        M.append(-(-len(rows) // P))

    MT, caps, fills = _plan(M)
    nseg = sum(1 for q in caps if q > 0)
    seg_base = np.cumsum([0] + list(caps)).tolist()

    x16 = x_flat.astype(np.float16)
    WeT16 = np.ascontiguousarray(We.transpose(0, 2, 1)).astype(np.float16)

    KT = CIN // P
    NT = SEG_COLS // N_TILE
    tok = np.zeros((NUM_CORES, MT, P), np.int64)  # token index per slot row
    scf = np.zeros((NUM_CORES, MT, P), np.float32)
    wts = np.zeros((NUM_CORES, nseg, NT, P, KT, N_TILE), np.float16)
    scatter = []  # (core, slot, e, h, count)
    for e, h, lo, hi, core, s in fills:
        rows = idx_e[e]
        ws = w_e[e]
        wts[core, s] = (
            WeT16[e][:, h * SEG_COLS : (h + 1) * SEG_COLS]
            .reshape(KT, P, NT, N_TILE)
            .transpose(2, 1, 0, 3)
        )
        for i, t in enumerate(range(lo, hi)):
            mslot = seg_base[s] + i
            sel = rows[t * P : (t + 1) * P]
            cnt = len(sel)
            if cnt == 0:
                continue
            tok[core, mslot, :cnt] = sel
            scf[core, mslot, :cnt] = ws[t * P : (t + 1) * P]
            scatter.append((core, mslot, e, h, cnt))

    in_maps = []
    for core in range(NUM_CORES):
        xg = x16[tok[core].reshape(-1)]  # [MT*128, CIN]
        xt = np.ascontiguousarray(
            xg.reshape(MT, P, KT, P).transpose(0, 3, 2, 1)
        ).reshape(MT, P, CIN)
        sc = np.ascontiguousarray(scf[core].T)  # [P, MT]
        in_maps.append({"xt": xt, "wt": wts[core], "sc": sc})

    key = (MT, caps, CIN)
    if key not in _NC_CACHE:
        _NC_CACHE[key] = _build_program(MT, caps, CIN)
    nc = _NC_CACHE[key]
    res = run_bass_kernel_spmd(nc, in_maps, list(range(NUM_CORES)), trace=TRACE)

    LAST_RUN_INFO.clear()
    LAST_RUN_INFO.update(
        exec_time_ns=res.exec_time_ns,
        mean_exec_time_ns=res.mean_exec_time_ns,
        max_exec_time_core_id=res.max_exec_time_core_id,
        profile_json=res.profile_json,
    )

    out = np.zeros((N, DOUT), np.float32)
    for core, mslot, e, h, cnt in scatter:
        ye = res.results[core]["y"][mslot, :cnt].astype(np.float32)
        rows = tok[core, mslot, :cnt]
        out[rows, h * SEG_COLS : (h + 1) * SEG_COLS] += ye
    return out.reshape(B, T, DOUT)


# revision 10
# speedup vs baseline: 1.0329x; 1.0329x over previous
"""MoE top-2 routed linear (nn_MoELinear) on 8 Trainium2 NeuronCores.

Strategy (load-balanced expert parallelism):
  - Gating (tiny: [N,1024]x[1024,8] matmul + top-2 + softmax) runs on host
    with jax-CPU, replicating the reference op-for-op so the top-2 decisions
    match the reference bitwise.
  - The per-expert work is split into "jobs" = (expert, 2048-col output
    half).  Each core holds up to 4 resident W segments of [1024, 2048]
    (fp16) in SBUF; a job is assigned to one or more (core, segment) cells,
    and its token tiles (128 tokens each) fill that cell's slot range.  A
    small search over segment capacities packs all jobs into the minimum
    uniform slot count MT per core (~ceil(2*sum(M_e)/8)), eliminating the
    load imbalance of the one-expert-per-core layout.
  - Device computes y[slot] = (x_tile @ Wseg) * gate_w in fp16 (fp32 PSUM
    accumulation), which measured at ~2e-3 max rel err for bf16 and better
    for fp16 -- far inside the 2e-2 gate.  y is written back in fp16.
  - Host scatters/accumulates the per-slot outputs into the final fp32
    result.
"""

import os

import numpy as np

NUM_CORES = 8
NUM_EXPERTS = 8
TOP_K = 2
P = 128  # partitions
N_TILE = 512  # psum free-dim tile (one bank of fp32)
SEG_COLS = 2048  # output columns per W segment (half of d_out)
MAX_SEGS = 4

# enable NTFF tracing (sets LAST_RUN_INFO["exec_time_ns"])
TRACE = os.environ.get("MOE_TRACE", "0") == "1"
MM_DTYPE = "float16"

LAST_RUN_INFO = {}
_NC_CACHE = {}


def _routing(x_flat, Wg, bg):
    """Replicate the reference gating bitwise on jax-CPU; numpy fallback."""
    try:
        import jax
        import jax.numpy as jnp

        with jax.default_device(jax.devices("cpu")[0]):
            xf = jnp.asarray(x_flat)
            gate_logits = xf @ jnp.asarray(Wg).T + jnp.asarray(bg)
            top_w, top_idx = jax.lax.top_k(gate_logits, TOP_K)
            top_w = jax.nn.softmax(top_w, axis=-1)
            return np.asarray(top_idx), np.asarray(top_w)
    except Exception:
        logits = x_flat @ Wg.T + bg
        top_idx = np.argsort(-logits, axis=1, kind="stable")[:, :TOP_K]
        top_v = np.take_along_axis(logits, top_idx, axis=1)
        e = np.exp(top_v - top_v.max(axis=1, keepdims=True))
        top_w = e / e.sum(axis=1, keepdims=True)
        return top_idx, top_w.astype(np.float32)


def _try_assign(jobs, caps):
    """Greedy pack of jobs (size, e, h) into cells of capacities caps x 8.

    Returns list of (e, h, tile_lo, tile_hi, core, seg) cell fills, or None.
    A job may span several cells (its tiles are split across them)."""
    cells = []  # [remaining?, core, seg, cap]
    for s, q in enumerate(caps):
        if q <= 0:
            continue
        for core in range(NUM_CORES):
            cells.append([q, core, s, False])  # cap, core, seg, used
    fills = []
    for size, e, h in jobs:
        free = [c for c in cells if not c[3]]
        # smallest single cell that fits
        fit = None
        for c in sorted(free, key=lambda c: c[0]):
            if c[0] >= size:
                fit = c
                break
        take = []
        if fit is not None:
            take = [fit]
        else:
            got = 0
            for c in sorted(free, key=lambda c: -c[0]):
                if c in take:
                    continue
                # if a single remaining cell can finish the cover, use the
                # smallest such cell instead of the next-largest
                fin = None
                for c2 in sorted(free, key=lambda c: c[0]):
                    if c2 not in take and c2[0] >= size - got:
                        fin = c2
                        break
                if fin is not None:
                    take.append(fin)
                    got += fin[0]
                    break
                take.append(c)
                got += c[0]
                if got >= size:
                    break
            if got < size:
                return None
        lo = 0
        for c in take:
            c[3] = True
            n = min(c[0], size - lo)
            fills.append((e, h, lo, lo + n, c[1], c[2]))
            lo += n
    return fills


def _plan(M):
    """Choose slot count MT, segment capacities, and the job->cell packing."""
    jobs = []
    for e in range(NUM_EXPERTS):
        if M[e] > 0:
            jobs.append((M[e], e, 0))
            jobs.append((M[e], e, 1))
    jobs.sort(reverse=True)
    total = sum(s for s, _, _ in jobs)
    lb = max(1, -(-total // NUM_CORES))
    for mt in range(lb, 4 * 64 + 1):
        for a in range(-(-mt // MAX_SEGS), mt + 1):
            for b in range(0, min(a, mt - a) + 1):
                rem = mt - a - b
                for c in range(max(0, rem - b), min(b, rem) + 1):
                    dd = rem - c
                    if dd > c:
                        continue
                    caps = (a, b, c, dd)
                    fills = _try_assign(jobs, caps)
                    if fills is not None:
                        return mt, caps, fills
    raise RuntimeError("no feasible packing")


def _build_program(MT, caps, CIN):
    """Uniform per-core program: MT slots, slot m uses W segment seg(m).

    y[m] = (x[m] @ Wseg) * sc[:, m] for its 2048 columns, fp16 I/O."""
    import concourse.mybir as mybir
    import concourse.tile as tile
    from concourse import bacc

    f32 = mybir.dt.float32
    f16 = mybir.dt.float16

    KT = CIN // P  # 8
    NT = SEG_COLS // N_TILE  # 4
    nseg = sum(1 for q in caps if q > 0)
    seg_of_slot = []
    for s, q in enumerate(caps):
        seg_of_slot += [s] * q

    nc = bacc.Bacc()
    # xt[m, p, k*128+j] = x[token (m,j), cin (k*128+p)]  (lhsT pretiled)
    xt = nc.declare_dram_parameter("xt", [MT, P, CIN], f16, isOutput=False)
    # wt[s, n, p, k, c] = W_seg_s.T[k*128+p, n*512+c]
    wt = nc.declare_dram_parameter(
        "wt", [nseg, NT, P, KT, N_TILE], f16, isOutput=False
    )
    # sc[p, m] = gate weight of token (m, p)
    sc = nc.declare_dram_parameter("sc", [P, MT], f32, isOutput=False)
    y = nc.declare_dram_parameter("y", [MT, P, SEG_COLS], f16, isOutput=True)

    PF = 6  # x-tile prefetch depth

    with tile.TileContext(nc) as tc:
        with (
            tc.tile_pool(name="wpool", bufs=1) as wpool,
            tc.tile_pool(name="xpool", bufs=min(PF + 2, MT)) as xpool,
            tc.tile_pool(name="spool", bufs=1) as spool,
            tc.tile_pool(name="opool", bufs=16) as opool,
            tc.tile_pool(name="pspool", bufs=8, space="PSUM") as pspool,
        ):
            sctile = spool.tile([P, MT], f32)

            # W segments as 3D tiles [p, k, 2048]; one 1MB descriptor per
            # (seg, n-quarter) keeps trigger count low (the trigger itself
            # costs ~600ns of engine time), except the very first n-group
            # which is split per-k so the first psum group can start after
            # ~128KB instead of 1MB.
            wtiles = [
                wpool.tile([P, KT, SEG_COLS], f16, tag=f"w{s}", name=f"w{s}")
                for s in range(nseg)
            ]

            def load_w(s, n, split=False):
                if split:
                    for k in range(KT):
                        nc.sync.dma_start(
                            out=wtiles[s][:, k, n * N_TILE : (n + 1) * N_TILE],
                            in_=wt[s, n, :, k],
                        )
                else:
                    nc.sync.dma_start(
                        out=wtiles[s][:, :, n * N_TILE : (n + 1) * N_TILE],
                        in_=wt[s, n],
                    )

            # x rides gpsimd (idle at start, later interleaves with the
            # y-store triggers); sync carries only W + scales so neither
            # stream queues behind the other.
            def load_xm(m):
                xtile = xpool.tile([P, CIN], f16, name="xtile", tag="xtile")
                nc.gpsimd.dma_start(out=xtile[:], in_=xt[m])
                return xtile

            load_w(0, 0, split=True)
            pending = [load_xm(m) for m in range(min(PF, MT))]
            nc.sync.dma_start(out=sctile[:], in_=sc[:, :])
            for n in range(1, NT):
                load_w(0, n)
            for s in range(1, nseg):
                for n in range(NT):
                    load_w(s, n)

            ydma = nc.gpsimd

            for m in range(MT):
                s = seg_of_slot[m]
                xtile = pending.pop(0)
                if m + PF < MT:
                    pending.append(load_xm(m + PF))
                stile = sctile[:, m : m + 1]
                for n in range(NT):
                    psum = pspool.tile([P, N_TILE], f32)
                    for k in range(KT):
                        nc.tensor.matmul(
                            psum[:],
                            lhsT=xtile[:, k * P : (k + 1) * P],
                            rhs=wtiles[s][:, k, n * N_TILE : (n + 1) * N_TILE],
                            start=(k == 0),
                            stop=(k == KT - 1),
                        )
                    otile = opool.tile([P, N_TILE], f16)
                    # alternate eviction engines: halves the serial latency
                    # of draining a slot's four psum banks
                    if n % 2 == 0:
                        nc.scalar.activation(
                            otile[:],
                            psum[:],
                            mybir.ActivationFunctionType.Copy,
                            scale=stile,
                        )
                    else:
                        nc.vector.tensor_scalar_mul(otile[:], psum[:], stile)
                    ydma.dma_start(
                        out=y[m, :, n * N_TILE : (n + 1) * N_TILE],
                        in_=otile[:],
                    )
    nc.finalize()
    return nc


def kernel(x, We, Wg, bg):
    from concourse.bass_utils import run_bass_kernel_spmd

    B, T, CIN = x.shape
    E, DOUT, _ = We.shape
    N = B * T
    x_flat = np.ascontiguousarray(x.reshape(N, CIN), dtype=np.float32)

    top_idx, top_w = _routing(x_flat, Wg, bg)

    # dispatch: token lists per expert
    idx_e, w_e, M = [], [], []
    for e in range(E):
        sel0 = top_idx[:, 0] == e
        sel1 = top_idx[:, 1] == e
        rows = np.nonzero(sel0 | sel1)[0]
        w = np.where(sel0[rows], top_w[rows, 0], top_w[rows, 1]).astype(np.float32)
        idx_e.append(rows)
        w_e.append(w)
        M.append(-(-len(rows) // P))

    MT, caps, fills = _plan(M)
    nseg = sum(1 for q in caps if q > 0)
    seg_base = np.cumsum([0] + list(caps)).tolist()

    x16 = x_flat.astype(np.float16)
    WeT16 = np.ascontiguousarray(We.transpose(0, 2, 1)).astype(np.float16)

    KT = CIN // P
    NT = SEG_COLS // N_TILE
    tok = np.zeros((NUM_CORES, MT, P), np.int64)  # token index per slot row
    scf = np.zeros((NUM_CORES, MT, P), np.float32)
    wts = np.zeros((NUM_CORES, nseg, NT, P, KT, N_TILE), np.float16)
    scatter = []  # (core, slot, e, h, count)
    for e, h, lo, hi, core, s in fills:
        rows = idx_e[e]
        ws = w_e[e]
        wts[core, s] = (
            WeT16[e][:, h * SEG_COLS : (h + 1) * SEG_COLS]
            .reshape(KT, P, NT, N_TILE)
            .transpose(2, 1, 0, 3)
        )
        for i, t in enumerate(range(lo, hi)):
            mslot = seg_base[s] + i
            sel = rows[t * P : (t + 1) * P]
            cnt = len(sel)
            if cnt == 0:
                continue
            tok[core, mslot, :cnt] = sel
            scf[core, mslot, :cnt] = ws[t * P : (t + 1) * P]
            scatter.append((core, mslot, e, h, cnt))

    in_maps = []
    for core in range(NUM_CORES):
        xg = x16[tok[core].reshape(-1)]  # [MT*128, CIN]
        xt = np.ascontiguousarray(
            xg.reshape(MT, P, KT, P).transpose(0, 3, 2, 1)
        ).reshape(MT, P, CIN)
        sc = np.ascontiguousarray(scf[core].T)  # [P, MT]
        in_maps.append({"xt": xt, "wt": wts[core], "sc": sc})

    key = (MT, caps, CIN)
    if key not in _NC_CACHE:
        _NC_CACHE[key] = _build_program(MT, caps, CIN)
    nc = _NC_CACHE[key]
    res = run_bass_kernel_spmd(nc, in_maps, list(range(NUM_CORES)), trace=TRACE)

    LAST_RUN_INFO.clear()
    LAST_RUN_INFO.update(
        exec_time_ns=res.exec_time_ns,
        mean_exec_time_ns=res.mean_exec_time_ns,
        max_exec_time_core_id=res.max_exec_time_core_id,
        profile_json=res.profile_json,
    )

    out = np.zeros((N, DOUT), np.float32)
    for core, mslot, e, h, cnt in scatter:
        ye = res.results[core]["y"][mslot, :cnt].astype(np.float32)
        rows = tok[core, mslot, :cnt]
        out[rows, h * SEG_COLS : (h + 1) * SEG_COLS] += ye
    return out.reshape(B, T, DOUT)


# revision 11
# speedup vs baseline: 1.0430x; 1.0098x over previous
"""MoE top-2 routed linear (nn_MoELinear) on 8 Trainium2 NeuronCores.

Strategy (load-balanced expert parallelism):
  - Gating (tiny: [N,1024]x[1024,8] matmul + top-2 + softmax) runs on host
    with jax-CPU, replicating the reference op-for-op so the top-2 decisions
    match the reference bitwise.
  - The per-expert work is split into "jobs" = (expert, 2048-col output
    half).  Each core holds up to 4 resident W segments of [1024, 2048]
    (fp16) in SBUF; a job is assigned to one or more (core, segment) cells,
    and its token tiles (128 tokens each) fill that cell's slot range.  A
    small search over segment capacities packs all jobs into the minimum
    uniform slot count MT per core (~ceil(2*sum(M_e)/8)), eliminating the
    load imbalance of the one-expert-per-core layout.
  - Device computes y[slot] = (x_tile @ Wseg) * gate_w in fp16 (fp32 PSUM
    accumulation), which measured at ~2e-3 max rel err for bf16 and better
    for fp16 -- far inside the 2e-2 gate.  y is written back in fp16.
  - Host scatters/accumulates the per-slot outputs into the final fp32
    result.
"""

import os

import numpy as np

NUM_CORES = 8
NUM_EXPERTS = 8
TOP_K = 2
P = 128  # partitions
N_TILE = 512  # psum free-dim tile (one bank of fp32)
SEG_COLS = 2048  # output columns per W segment (half of d_out)
MAX_SEGS = 4

# enable NTFF tracing (sets LAST_RUN_INFO["exec_time_ns"])
TRACE = os.environ.get("MOE_TRACE", "0") == "1"
MM_DTYPE = "float16"

LAST_RUN_INFO = {}
_NC_CACHE = {}


def _routing(x_flat, Wg, bg):
    """Replicate the reference gating bitwise on jax-CPU; numpy fallback."""
    try:
        import jax
        import jax.numpy as jnp

        with jax.default_device(jax.devices("cpu")[0]):
            xf = jnp.asarray(x_flat)
            gate_logits = xf @ jnp.asarray(Wg).T + jnp.asarray(bg)
            top_w, top_idx = jax.lax.top_k(gate_logits, TOP_K)
            top_w = jax.nn.softmax(top_w, axis=-1)
            return np.asarray(top_idx), np.asarray(top_w)
    except Exception:
        logits = x_flat @ Wg.T + bg
        top_idx = np.argsort(-logits, axis=1, kind="stable")[:, :TOP_K]
        top_v = np.take_along_axis(logits, top_idx, axis=1)
        e = np.exp(top_v - top_v.max(axis=1, keepdims=True))
        top_w = e / e.sum(axis=1, keepdims=True)
        return top_idx, top_w.astype(np.float32)


def _try_assign(jobs, caps):
    """Greedy pack of jobs (size, e, h) into cells of capacities caps x 8.

    Returns list of (e, h, tile_lo, tile_hi, core, seg) cell fills, or None.
    A job may span several cells (its tiles are split across them)."""
    cells = []  # [remaining?, core, seg, cap]
    for s, q in enumerate(caps):
        if q <= 0:
            continue
        for core in range(NUM_CORES):
            cells.append([q, core, s, False])  # cap, core, seg, used
    fills = []
    for size, e, h in jobs:
        free = [c for c in cells if not c[3]]
        # smallest single cell that fits
        fit = None
        for c in sorted(free, key=lambda c: c[0]):
            if c[0] >= size:
                fit = c
                break
        take = []
        if fit is not None:
            take = [fit]
        else:
            got = 0
            for c in sorted(free, key=lambda c: -c[0]):
                if c in take:
                    continue
                # if a single remaining cell can finish the cover, use the
                # smallest such cell instead of the next-largest
                fin = None
                for c2 in sorted(free, key=lambda c: c[0]):
                    if c2 not in take and c2[0] >= size - got:
                        fin = c2
                        break
                if fin is not None:
                    take.append(fin)
                    got += fin[0]
                    break
                take.append(c)
                got += c[0]
                if got >= size:
                    break
            if got < size:
                return None
        lo = 0
        for c in take:
            c[3] = True
            n = min(c[0], size - lo)
            fills.append((e, h, lo, lo + n, c[1], c[2]))
            lo += n
    return fills


def _plan(M):
    """Choose slot count MT, segment capacities, and the job->cell packing."""
    jobs = []
    for e in range(NUM_EXPERTS):
        if M[e] > 0:
            jobs.append((M[e], e, 0))
            jobs.append((M[e], e, 1))
    jobs.sort(reverse=True)
    total = sum(s for s, _, _ in jobs)
    lb = max(1, -(-total // NUM_CORES))
    for mt in range(lb, 4 * 64 + 1):
        for a in range(-(-mt // MAX_SEGS), mt + 1):
            for b in range(0, min(a, mt - a) + 1):
                rem = mt - a - b
                for c in range(max(0, rem - b), min(b, rem) + 1):
                    dd = rem - c
                    if dd > c:
                        continue
                    caps = (a, b, c, dd)
                    fills = _try_assign(jobs, caps)
                    if fills is not None:
                        return mt, caps, fills
    raise RuntimeError("no feasible packing")


def _build_program(MT, caps, CIN):
    """Uniform per-core program: MT slots, slot m uses W segment seg(m).

    y[m] = (x[m] @ Wseg) * sc[:, m] for its 2048 columns, fp16 I/O."""
    import concourse.mybir as mybir
    import concourse.tile as tile
    from concourse import bacc

    f32 = mybir.dt.float32
    f16 = mybir.dt.float16

    KT = CIN // P  # 8
    NT = SEG_COLS // N_TILE  # 4
    nseg = sum(1 for q in caps if q > 0)
    seg_of_slot = []
    for s, q in enumerate(caps):
        seg_of_slot += [s] * q

    nc = bacc.Bacc()
    # xt[m, p, k*128+j] = x[token (m,j), cin (k*128+p)]  (lhsT pretiled)
    xt = nc.declare_dram_parameter("xt", [MT, P, CIN], f16, isOutput=False)
    # wt[s, n, p, k, c] = W_seg_s.T[k*128+p, n*512+c]
    wt = nc.declare_dram_parameter(
        "wt", [nseg, NT, P, KT, N_TILE], f16, isOutput=False
    )
    # sc[p, m] = gate weight of token (m, p)
    sc = nc.declare_dram_parameter("sc", [P, MT], f32, isOutput=False)
    y = nc.declare_dram_parameter("y", [MT, P, SEG_COLS], f16, isOutput=True)

    PF = 6  # x-tile prefetch depth

    with tile.TileContext(nc) as tc:
        with (
            tc.tile_pool(name="wpool", bufs=1) as wpool,
            tc.tile_pool(name="xpool", bufs=min(PF + 2, MT)) as xpool,
            tc.tile_pool(name="spool", bufs=1) as spool,
            tc.tile_pool(name="opool", bufs=16) as opool,
            tc.tile_pool(name="pspool", bufs=8, space="PSUM") as pspool,
        ):
            sctile = spool.tile([P, MT], f32)

            # W segments as 3D tiles [p, k, 2048]; one 1MB descriptor per
            # (seg, n-quarter) keeps trigger count low (the trigger itself
            # costs ~600ns of engine time), except the very first n-group
            # which is split per-k so the first psum group can start after
            # ~128KB instead of 1MB.
            wtiles = [
                wpool.tile([P, KT, SEG_COLS], f16, tag=f"w{s}", name=f"w{s}")
                for s in range(nseg)
            ]

            def load_w(s, n, split=False):
                if split:
                    for k in range(KT):
                        nc.sync.dma_start(
                            out=wtiles[s][:, k, n * N_TILE : (n + 1) * N_TILE],
                            in_=wt[s, n, :, k],
                        )
                else:
                    nc.sync.dma_start(
                        out=wtiles[s][:, :, n * N_TILE : (n + 1) * N_TILE],
                        in_=wt[s, n],
                    )

            # x rides gpsimd (idle at start, later interleaves with the
            # y-store triggers); sync carries only W + scales so neither
            # stream queues behind the other.
            def load_xm(m):
                xtile = xpool.tile([P, CIN], f16, name="xtile", tag="xtile")
                nc.gpsimd.dma_start(out=xtile[:], in_=xt[m])
                return xtile

            load_w(0, 0, split=True)
            pending = [load_xm(m) for m in range(min(PF, MT))]
            nc.sync.dma_start(out=sctile[:], in_=sc[:, :])
            for n in range(1, NT):
                load_w(0, n, split=True)
            for s in range(1, nseg):
                for n in range(NT):
                    load_w(s, n)

            ydma = nc.scalar

            for m in range(MT):
                s = seg_of_slot[m]
                xtile = pending.pop(0)
                if m + PF < MT:
                    pending.append(load_xm(m + PF))
                stile = sctile[:, m : m + 1]
                for n in range(NT):
                    psum = pspool.tile([P, N_TILE], f32)
                    for k in range(KT):
                        nc.tensor.matmul(
                            psum[:],
                            lhsT=xtile[:, k * P : (k + 1) * P],
                            rhs=wtiles[s][:, k, n * N_TILE : (n + 1) * N_TILE],
                            start=(k == 0),
                            stop=(k == KT - 1),
                        )
                    otile = opool.tile([P, N_TILE], f16)
                    # alternate eviction engines: halves the serial latency
                    # of draining a slot's four psum banks
                    if n % 2 == 0:
                        nc.scalar.activation(
                            otile[:],
                            psum[:],
                            mybir.ActivationFunctionType.Copy,
                            scale=stile,
                        )
                    else:
                        nc.vector.tensor_scalar_mul(otile[:], psum[:], stile)
                    ydma.dma_start(
                        out=y[m, :, n * N_TILE : (n + 1) * N_TILE],
                        in_=otile[:],
                    )
    nc.finalize()
    return nc


def kernel(x, We, Wg, bg):
    from concourse.bass_utils import run_bass_kernel_spmd

    B, T, CIN = x.shape
    E, DOUT, _ = We.shape
    N = B * T
    x_flat = np.ascontiguousarray(x.reshape(N, CIN), dtype=np.float32)

    top_idx, top_w = _routing(x_flat, Wg, bg)

    # dispatch: token lists per expert
    idx_e, w_e, M = [], [], []
    for e in range(E):
        sel0 = top_idx[:, 0] == e
        sel1 = top_idx[:, 1] == e
        rows = np.nonzero(sel0 | sel1)[0]
        w = np.where(sel0[rows], top_w[rows, 0], top_w[rows, 1]).astype(np.float32)
        idx_e.append(rows)
        w_e.append(w)
        M.append(-(-len(rows) // P))

    MT, caps, fills = _plan(M)
    nseg = sum(1 for q in caps if q > 0)
    seg_base = np.cumsum([0] + list(caps)).tolist()

    x16 = x_flat.astype(np.float16)
    WeT16 = np.ascontiguousarray(We.transpose(0, 2, 1)).astype(np.float16)

    KT = CIN // P
    NT = SEG_COLS // N_TILE
    tok = np.zeros((NUM_CORES, MT, P), np.int64)  # token index per slot row
    scf = np.zeros((NUM_CORES, MT, P), np.float32)
    wts = np.zeros((NUM_CORES, nseg, NT, P, KT, N_TILE), np.float16)
    scatter = []  # (core, slot, e, h, count)
    for e, h, lo, hi, core, s in fills:
        rows = idx_e[e]
        ws = w_e[e]
        wts[core, s] = (
            WeT16[e][:, h * SEG_COLS : (h + 1) * SEG_COLS]
            .reshape(KT, P, NT, N_TILE)
            .transpose(2, 1, 0, 3)
        )
        for i, t in enumerate(range(lo, hi)):
            mslot = seg_base[s] + i
            sel = rows[t * P : (t + 1) * P]
            cnt = len(sel)
            if cnt == 0:
                continue
            tok[core, mslot, :cnt] = sel
            scf[core, mslot, :cnt] = ws[t * P : (t + 1) * P]
            scatter.append((core, mslot, e, h, cnt))

    in_maps = []
    for core in range(NUM_CORES):
        xg = x16[tok[core].reshape(-1)]  # [MT*128, CIN]
        xt = np.ascontiguousarray(
            xg.reshape(MT, P, KT, P).transpose(0, 3, 2, 1)
        ).reshape(MT, P, CIN)
        sc = np.ascontiguousarray(scf[core].T)  # [P, MT]
        in_maps.append({"xt": xt, "wt": wts[core], "sc": sc})

    key = (MT, caps, CIN)
    if key not in _NC_CACHE:
        _NC_CACHE[key] = _build_program(MT, caps, CIN)
    nc = _NC_CACHE[key]
    res = run_bass_kernel_spmd(nc, in_maps, list(range(NUM_CORES)), trace=TRACE)

    LAST_RUN_INFO.clear()
    LAST_RUN_INFO.update(
        exec_time_ns=res.exec_time_ns,
        mean_exec_time_ns=res.mean_exec_time_ns,
        max_exec_time_core_id=res.max_exec_time_core_id,
        profile_json=res.profile_json,
    )

    out = np.zeros((N, DOUT), np.float32)
    for core, mslot, e, h, cnt in scatter:
        ye = res.results[core]["y"][mslot, :cnt].astype(np.float32)
        rows = tok[core, mslot, :cnt]
        out[rows, h * SEG_COLS : (h + 1) * SEG_COLS] += ye
    return out.reshape(B, T, DOUT)
